# revision 24
# baseline (speedup 1.0000x reference)
"""Distributed causal-attention block (dense_transformer) on 8 TRN2 NeuronCores.

Sharding: data-parallel over batch (b=2) x tensor-parallel over head pairs
(8 heads -> 4 groups of 2). Core i handles batch i//4, heads (2*(i%4), 2*(i%4)+1).
Per-core: QKV projection for its 2 heads (transposed layouts so attention is
transpose-free), block-causal flash-style attention (S^T = K @ Q^T formulation,
softmax denominator via an augmented ones-column in V), partial output
projection, then ReduceScatter(add) over each 4-core batch group.

Software pipelining: projections for q tile t+1 and the output gather for
tile t-4 are emitted as small work units inside tile t's k-block loop,
between the s matmul and PV — the tensor engine fills that slot while the
scalar engine runs exp, so neither engine starves at tile boundaries.

B, S, D, H = 2, 4096, 512, 8 (hd=64). Hardcoded per problem spec.
"""

import numpy as np
import ml_dtypes

import concourse.bacc as bacc
import concourse.mybir as mybir
from concourse import tile
from concourse.bass_utils import run_bass_kernel_spmd

B, S, D = 2, 4096, 512
H = 8
HD = D // H          # 64
NCORES = 8
R = 128              # qkv rows per core (2 heads x 64)
S4 = S // 4          # reduce-scatter shard rows
NT = 8               # q tiles of 512
QW = 512             # q tile width

BF16 = mybir.dt.bfloat16
F32 = mybir.dt.float32
AF = mybir.ActivationFunctionType
BF16_NP = ml_dtypes.bfloat16

_CACHE = {}


def _build_nc():
    nc = bacc.Bacc(num_devices=NCORES)

    xT = nc.declare_dram_parameter("xT", [D, S], BF16, isOutput=False)
    wqT = nc.declare_dram_parameter("wqT", [D, R], BF16, isOutput=False)
    wkT = nc.declare_dram_parameter("wkT", [D, R], BF16, isOutput=False)
    wvT = nc.declare_dram_parameter("wvT", [D, 130], BF16, isOutput=False)
    bq = nc.declare_dram_parameter("bq", [R, 1], F32, isOutput=False)
    bk = nc.declare_dram_parameter("bk", [R, 1], F32, isOutput=False)
    bvb = nc.declare_dram_parameter("bvb", [128, 130], F32, isOutput=False)
    wo0 = nc.declare_dram_parameter("wo0", [HD, D], BF16, isOutput=False)
    wo1 = nc.declare_dram_parameter("wo1", [HD, D], BF16, isOutput=False)
    bob4 = nc.declare_dram_parameter("bob4", [128, D], F32, isOutput=False)
    maskc = nc.declare_dram_parameter("maskc", [128, 4 * QW], BF16, isOutput=False)
    out_ext = nc.declare_dram_parameter("out", [S4, D], F32, isOutput=True)

    parts = [nc.dram_tensor(f"part{t}", [QW, D], BF16) for t in range(NT)]
    ldram = [nc.dram_tensor(f"ldram{t}", [2 * QW], F32) for t in range(NT)]
    rss = [nc.dram_tensor(f"rs{t}", [QW // 4, D], BF16) for t in range(NT)]

    with tile.TileContext(nc) as tc:
        with (
            tc.tile_pool(name="const", bufs=1) as cpool,
            tc.tile_pool(name="xres", bufs=1) as xpool,
            tc.tile_pool(name="pt", bufs=24) as ppool,
            tc.tile_pool(name="small", bufs=3) as spool,
            tc.tile_pool(name="stage", bufs=4) as stpool,
            tc.tile_pool(name="ps_s", bufs=2, space="PSUM") as ps_s,
            tc.tile_pool(name="ps_o", bufs=1, space="PSUM") as ps_o,
        ):
            # ---------- loads in first-use order: x first halves, QKV
            # weights, x second halves, then the epilogue constants
            xt = [xpool.tile([128, S], BF16, tag=f"xt{c}", name=f"xt{c}")
                  for c in range(4)]
            qT = xpool.tile([128, S], BF16, tag="qT")
            kT = xpool.tile([128, S], BF16, tag="kT")
            vaug = xpool.tile([128, 32 * 130], BF16, tag="vaug")

            def load_x(c, half):
                nc.sync.dma_start(
                    xt[c][:, (S // 2) * half:(S // 2) * (half + 1)],
                    xT[128 * c:128 * (c + 1),
                       (S // 2) * half:(S // 2) * (half + 1)],
                )

            for c in range(4):
                load_x(c, 0)
            wq_sb = cpool.tile([128, D], BF16)
            nc.sync.dma_start(wq_sb[:].rearrange("p (c m) -> p c m", c=4),
                              wqT[:, :].rearrange("(c p) m -> p c m", p=128))
            wk_sb = cpool.tile([128, D], BF16)
            nc.sync.dma_start(wk_sb[:].rearrange("p (c m) -> p c m", c=4),
                              wkT[:, :].rearrange("(c p) m -> p c m", p=128))
            wv_sb = cpool.tile([128, 4 * 130], BF16)
            nc.sync.dma_start(wv_sb[:].rearrange("p (c m) -> p c m", c=4),
                              wvT[:, :].rearrange("(c p) m -> p c m", p=128))
            bq_sb = cpool.tile([R, 1], F32)
            nc.sync.dma_start(bq_sb[:], bq[:, :])
            bk_sb = cpool.tile([R, 1], F32)
            nc.sync.dma_start(bk_sb[:], bk[:, :])
            bvb_sb = cpool.tile([128, 130], F32)
            nc.sync.dma_start(bvb_sb[:], bvb[:, :])
            mask_sb = cpool.tile([128, 4 * QW], BF16)
            nc.sync.dma_start(mask_sb[:], maskc[:, :])
            for c in range(4):
                load_x(c, 1)
            wo0_sb = cpool.tile([HD, D], BF16)
            nc.sync.dma_start(wo0_sb[:], wo0[:, :])
            wo1_sb = cpool.tile([HD, D], BF16)
            nc.sync.dma_start(wo1_sb[:], wo1[:, :])
            bob4_sb = cpool.tile([128, D], F32)
            nc.sync.dma_start(bob4_sb[:], bob4[:, :])

            def proj_units(nt):
                # 6 small units (q, k, 4x v) for token block nt
                def qk_unit(w_sb, b_sb, dst, tag):
                    def u():
                        ps = ps_o.tile([128, QW], F32, tag=tag, name="ps_qk")
                        for c in range(4):
                            nc.tensor.matmul(
                                ps[:],
                                w_sb[:, 128 * c:128 * (c + 1)],
                                xt[c][:, QW * nt:QW * (nt + 1)],
                                start=(c == 0), stop=(c == 3),
                            )
                        nc.vector.tensor_scalar_add(
                            dst[:, QW * nt:QW * (nt + 1)], ps[:], b_sb[:])
                    return u

                def v_unit(tb):
                    def u():
                        ps = ps_o.tile([128, QW], F32, tag=f"po{tb % 2}",
                                       name="ps_v")
                        for c in range(4):
                            nc.tensor.matmul(
                                ps[:, 0:130],
                                xt[c][:, 128 * tb:128 * (tb + 1)],
                                wv_sb[:, 130 * c:130 * (c + 1)],
                                start=(c == 0), stop=(c == 3),
                            )
                        nc.vector.tensor_add(
                            vaug[:, 130 * tb:130 * (tb + 1)], ps[:, 0:130],
                            bvb_sb[:])
                    return u

                return ([qk_unit(wq_sb, bq_sb, qT, "po0"),
                         qk_unit(wk_sb, bk_sb, kT, "po1")]
                        + [v_unit(tb) for tb in range(4 * nt, 4 * (nt + 1))])

            def gather_unit(tg):
                # rs -> sbuf, cast bf16 -> f32, write the output rows
                def u():
                    g = stpool.tile([128, D], BF16, tag="g", name="g")
                    nc.sync.dma_start(g[:], rss[tg][:, :])
                    gf = stpool.tile([128, D], F32, tag="gf", name="gf")
                    nc.vector.tensor_copy(gf[:], g[:])
                    nc.sync.dma_start(out_ext[128 * tg:128 * (tg + 1), :], gf[:])
                return u

            def att_jloop(t, units):
                nj = 4 * t + 4          # causal: k blocks 0 .. 4t+3
                o0 = ps_o.tile([65, QW], F32, tag="o0", name="o0")
                o1 = ps_o.tile([65, QW], F32, tag="o1", name="o1")
                n_units = len(units)
                emitted = 0
                for j in range(nj):
                    # causal: q columns < q0 are fully masked for this k block
                    q0 = max(0, 128 * (j - 4 * t))
                    s = ps_s.tile([128, 2 * QW], F32, tag="s", name="s")
                    for h in (0, 1):
                        nc.tensor.matmul(
                            s[:, QW * h + q0:QW * (h + 1)],
                            kT[64 * h:64 * (h + 1), 128 * j:128 * (j + 1)],
                            qT[64 * h:64 * (h + 1), QW * t + q0:QW * (t + 1)],
                            start=True, stop=True,
                        )
                    p = ppool.tile([128, 2 * QW], BF16, tag="p", name="p")
                    if q0 == 0:
                        nc.scalar.activation(p[:], s[:], AF.Exp, bias=0.0, scale=0.125)
                    else:
                        sv = s[:].rearrange("k (h q) -> k h q", h=2)[:, :, q0:QW]
                        pv = p[:].rearrange("k (h q) -> k h q", h=2)[:, :, q0:QW]
                        nc.scalar.activation(pv, sv, AF.Exp, bias=0.0, scale=0.125)
                    while emitted < n_units and emitted <= j * n_units // nj:
                        units[emitted]()
                        emitted += 1
                    if j >= 4 * t:  # diagonal 128-col boundary: 0/1 mask (r=0 tile)
                        for h in (0, 1):
                            nc.gpsimd.tensor_mul(
                                p[:, QW * h + q0:QW * h + q0 + 128],
                                p[:, QW * h + q0:QW * h + q0 + 128],
                                mask_sb[:, 0:128],
                            )
                    for h, oo in ((0, o0), (1, o1)):
                        nc.tensor.matmul(
                            oo[:, q0:QW],
                            vaug[:, 130 * j + 65 * h:130 * j + 65 * (h + 1)],
                            p[:, QW * h + q0:QW * (h + 1)],
                            start=(j == 0), stop=(j == nj - 1),
                        )
                for u in units[emitted:]:
                    u()
                return o0, o1

            def epilogue(t, o0, o1):
                # softmax denominators (row 64 of o0/o1), normalize, O-proj.
                # free the o accumulators fast: copy O^T (rows 0..63) to SBUF
                # and the l rows (row 64) to an SBUF staging row.
                oc0 = spool.tile([HD, QW], BF16, tag="oc0", name="oc0")
                oc1 = spool.tile([HD, QW], BF16, tag="oc1", name="oc1")
                nc.vector.tensor_copy(oc0[:], o0[0:64, :])
                nc.vector.tensor_copy(oc1[:], o1[0:64, :])
                lrow = spool.tile([128, 2 * QW], F32, tag="lrow", name="lrow")
                nc.vector.tensor_copy(lrow[64:65, 0:QW], o0[64:65, :])
                nc.vector.tensor_copy(lrow[64:65, QW:2 * QW], o1[64:65, :])

                # l -> natural per-token-partition layout [128, tb] via DMA,
                # then reciprocal (base-0 only: reciprocal_approx_fast quirk)
                l_nat = spool.tile([128, 8], F32, tag="l_nat", name="l_nat")
                nc.sync.dma_start(ldram[t][:].rearrange("(a q) -> a q", a=1),
                                  lrow[64:65, :])
                for h in range(2):
                    nc.sync.dma_start(
                        l_nat[:, 4 * h:4 * (h + 1)],
                        ldram[t][QW * h:QW * (h + 1)].rearrange(
                            "(tb p) -> p tb", p=128),
                    )
                linv_nat = spool.tile([128, 8], F32, tag="linv_nat", name="linv")
                nc.vector.reciprocal_approx_fast(linv_nat[:], l_nat[:])

                for tb in range(4):
                    po0 = ps_o.tile([128, QW], F32, tag="po0", name="po0")
                    po1 = ps_o.tile([128, QW], F32, tag="po1", name="po1")
                    nc.tensor.matmul(po0[:], oc0[:, 128 * tb:128 * (tb + 1)],
                                     wo0_sb[:], start=True, stop=True)
                    nc.tensor.matmul(po1[:], oc1[:, 128 * tb:128 * (tb + 1)],
                                     wo1_sb[:], start=True, stop=True)
                    tmp = stpool.tile([128, QW], F32, tag="tmp", name="tmp")
                    nc.vector.scalar_tensor_tensor(
                        tmp[:], po0[:], linv_nat[:, tb:tb + 1], bob4_sb[:],
                        mybir.AluOpType.mult, mybir.AluOpType.add)
                    st = stpool.tile([128, QW], BF16, tag="st", name="st")
                    nc.vector.scalar_tensor_tensor(
                        st[:], po1[:], linv_nat[:, 4 + tb:5 + tb], tmp[:],
                        mybir.AluOpType.mult, mybir.AluOpType.add)
                    nc.sync.dma_start(
                        parts[t][128 * tb:128 * (tb + 1), :], st[:]
                    )
                # chunked ReduceScatter: overlaps with later q tiles
                nc.gpsimd.collective_compute(
                    "ReduceScatter",
                    mybir.AluOpType.add,
                    replica_groups=[[0, 1, 2, 3], [4, 5, 6, 7]],
                    ins=[parts[t][:, :]],
                    outs=[rss[t][:, :]],
                )

            for u in proj_units(0):
                u()
            for t in range(NT):
                units = []
                if t >= 4:
                    units.append(gather_unit(t - 4))
                if t + 1 < NT:
                    units.extend(proj_units(t + 1))
                o0, o1 = att_jloop(t, units)
                epilogue(t, o0, o1)
            for tg in range(NT - 4, NT):
                gather_unit(tg)()

    nc.finalize()
    return nc


def _make_in_maps(x, Wqkv, bqkv, Wo, bo):
    # causal 0/1 multiplicative masks for the 4 diagonal sub-block offsets:
    # keep (p, o) where o >= 128*r + p  (k = 128*(4t+r)+p, q = 512*t+o)
    p_idx = np.arange(128)[:, None]
    o_idx = np.arange(QW)[None, :]
    maskc = np.concatenate(
        [(o_idx >= 128 * r + p_idx).astype(np.float32) for r in range(4)], axis=1
    ).astype(BF16_NP)

    in_maps = []
    for core in range(NCORES):
        b = core // 4
        g = core % 4
        rows = slice(128 * g, 128 * (g + 1))
        wq = Wqkv[0:D][rows]            # [128, 512]
        wk = Wqkv[D:2 * D][rows]
        wv = Wqkv[2 * D:3 * D][rows]
        wvT = np.zeros((D, 130), dtype=np.float32)
        wvT[:, 0:64] = wv[0:64].T
        wvT[:, 65:129] = wv[64:128].T
        bvb = np.zeros((128, 130), dtype=np.float32)
        bvb[:, 0:64] = bqkv[2 * D:3 * D][rows][0:64][None, :]
        bvb[:, 64] = 1.0
        bvb[:, 65:129] = bqkv[2 * D:3 * D][rows][64:128][None, :]
        bvb[:, 129] = 1.0
        in_maps.append({
            "xT": np.ascontiguousarray(x[b].T).astype(BF16_NP),
            "wqT": np.ascontiguousarray(wq.T).astype(BF16_NP),
            "wkT": np.ascontiguousarray(wk.T).astype(BF16_NP),
            "wvT": wvT.astype(BF16_NP),
            "bq": np.ascontiguousarray(bqkv[0:D][rows][:, None]).astype(np.float32),
            "bk": np.ascontiguousarray(bqkv[D:2 * D][rows][:, None]).astype(np.float32),
            "bvb": bvb,
            "wo0": np.ascontiguousarray(Wo[:, 128 * g:128 * g + 64].T).astype(BF16_NP),
            "wo1": np.ascontiguousarray(Wo[:, 128 * g + 64:128 * (g + 1)].T).astype(BF16_NP),
            "bob4": np.tile((bo / 4.0).astype(np.float32)[None, :], (128, 1)),
            "maskc": maskc,
        })
    return in_maps


def run(x, Wqkv, bqkv, Wo, bo, trace=False):
    if "nc" not in _CACHE:
        _CACHE["nc"] = _build_nc()
    nc = _CACHE["nc"]
    in_maps = _make_in_maps(x, Wqkv, bqkv, Wo, bo)
    res = run_bass_kernel_spmd(nc, in_maps, core_ids=list(range(NCORES)), trace=trace)
    out = np.empty((B, S, D), dtype=np.float32)
    for core in range(NCORES):
        b = core // 4
        r = core % 4
        o = res.results[core]["out"]
        # chunked ReduceScatter: rank r holds rows 512t+128r..+128 per q tile t
        for t in range(NT):
            out[b, QW * t + 128 * r:QW * t + 128 * (r + 1), :] = o[128 * t:128 * (t + 1)]
    return out, res


def kernel(x, Wqkv, bqkv, Wo, bo):
    out, _ = run(np.asarray(x, dtype=np.float32), np.asarray(Wqkv, dtype=np.float32),
                 np.asarray(bqkv, dtype=np.float32), np.asarray(Wo, dtype=np.float32),
                 np.asarray(bo, dtype=np.float32))
    return out


# revision 25
# speedup vs baseline: 1.0531x; 1.0531x over previous
"""Distributed causal-attention block (dense_transformer) on 8 TRN2 NeuronCores.

Sharding: data-parallel over batch (b=2) x tensor-parallel over head pairs
(8 heads -> 4 groups of 2). Core i handles batch i//4, heads (2*(i%4), 2*(i%4)+1).
Per-core: QKV projection for its 2 heads (transposed layouts so attention is
transpose-free), block-causal flash-style attention (S^T = K @ Q^T formulation,
softmax denominator via an augmented ones-column in V), partial output
projection, then ReduceScatter(add) over each 4-core batch group.

Software pipelining: projections for q tile t+1 and the output gather for
tile t-4 are emitted as small work units inside tile t's k-block loop,
between the s matmul and PV — the tensor engine fills that slot while the
scalar engine runs exp, so neither engine starves at tile boundaries.

B, S, D, H = 2, 4096, 512, 8 (hd=64). Hardcoded per problem spec.
"""

import numpy as np
import ml_dtypes

import concourse.bacc as bacc
import concourse.mybir as mybir
from concourse import tile
from concourse.bass_utils import run_bass_kernel_spmd

B, S, D = 2, 4096, 512
H = 8
HD = D // H          # 64
NCORES = 8
R = 128              # qkv rows per core (2 heads x 64)
S4 = S // 4          # reduce-scatter shard rows
NT = 8               # q tiles of 512
QW = 512             # q tile width

BF16 = mybir.dt.bfloat16
F32 = mybir.dt.float32
AF = mybir.ActivationFunctionType
BF16_NP = ml_dtypes.bfloat16

_CACHE = {}


def _build_nc():
    nc = bacc.Bacc(num_devices=NCORES)

    xT = nc.declare_dram_parameter("xT", [D, S], BF16, isOutput=False)
    wqT = nc.declare_dram_parameter("wqT", [D, R], BF16, isOutput=False)
    wkT = nc.declare_dram_parameter("wkT", [D, R], BF16, isOutput=False)
    wvT = nc.declare_dram_parameter("wvT", [D, 130], BF16, isOutput=False)
    bq = nc.declare_dram_parameter("bq", [R, 1], F32, isOutput=False)
    bk = nc.declare_dram_parameter("bk", [R, 1], F32, isOutput=False)
    bvb = nc.declare_dram_parameter("bvb", [128, 130], F32, isOutput=False)
    wo0 = nc.declare_dram_parameter("wo0", [HD, D], BF16, isOutput=False)
    wo1 = nc.declare_dram_parameter("wo1", [HD, D], BF16, isOutput=False)
    bob4 = nc.declare_dram_parameter("bob4", [128, D], F32, isOutput=False)
    maskc = nc.declare_dram_parameter("maskc", [128, 4 * QW], BF16, isOutput=False)
    out_ext = nc.declare_dram_parameter("out", [S4, D], F32, isOutput=True)

    parts = [nc.dram_tensor(f"part{t}", [QW, D], BF16) for t in range(NT)]
    ldram = [nc.dram_tensor(f"ldram{t}", [2 * QW], F32) for t in range(NT)]
    rss = [nc.dram_tensor(f"rs{t}", [QW // 4, D], BF16) for t in range(NT)]

    with tile.TileContext(nc) as tc:
        with (
            tc.tile_pool(name="const", bufs=1) as cpool,
            tc.tile_pool(name="xres", bufs=1) as xpool,
            tc.tile_pool(name="pt", bufs=24) as ppool,
            tc.tile_pool(name="small", bufs=3) as spool,
            tc.tile_pool(name="stage", bufs=4) as stpool,
            tc.tile_pool(name="ps_s", bufs=2, space="PSUM") as ps_s,
            tc.tile_pool(name="ps_o", bufs=1, space="PSUM") as ps_o,
        ):
            # ---------- loads in first-use order: x first halves, QKV
            # weights, x second halves, then the epilogue constants
            xt = [xpool.tile([128, S], BF16, tag=f"xt{c}", name=f"xt{c}")
                  for c in range(4)]
            qT = xpool.tile([128, S], BF16, tag="qT")
            kT = xpool.tile([128, S], BF16, tag="kT")
            vaug = xpool.tile([128, 32 * 130], BF16, tag="vaug")

            def load_x(c, half):
                nc.sync.dma_start(
                    xt[c][:, (S // 2) * half:(S // 2) * (half + 1)],
                    xT[128 * c:128 * (c + 1),
                       (S // 2) * half:(S // 2) * (half + 1)],
                )

            for c in range(4):
                load_x(c, 0)
            wq_sb = cpool.tile([128, D], BF16)
            nc.sync.dma_start(wq_sb[:].rearrange("p (c m) -> p c m", c=4),
                              wqT[:, :].rearrange("(c p) m -> p c m", p=128))
            wk_sb = cpool.tile([128, D], BF16)
            nc.sync.dma_start(wk_sb[:].rearrange("p (c m) -> p c m", c=4),
                              wkT[:, :].rearrange("(c p) m -> p c m", p=128))
            wv_sb = cpool.tile([128, 4 * 130], BF16)
            nc.sync.dma_start(wv_sb[:].rearrange("p (c m) -> p c m", c=4),
                              wvT[:, :].rearrange("(c p) m -> p c m", p=128))
            bq_sb = cpool.tile([R, 1], F32)
            nc.sync.dma_start(bq_sb[:], bq[:, :])
            bk_sb = cpool.tile([R, 1], F32)
            nc.sync.dma_start(bk_sb[:], bk[:, :])
            bvb_sb = cpool.tile([128, 130], F32)
            nc.sync.dma_start(bvb_sb[:], bvb[:, :])
            mask_sb = cpool.tile([128, 4 * QW], BF16)
            nc.sync.dma_start(mask_sb[:], maskc[:, :])
            for c in range(4):
                load_x(c, 1)
            wo0_sb = cpool.tile([HD, D], BF16)
            nc.sync.dma_start(wo0_sb[:], wo0[:, :])
            wo1_sb = cpool.tile([HD, D], BF16)
            nc.sync.dma_start(wo1_sb[:], wo1[:, :])
            bob4_sb = cpool.tile([128, D], F32)
            nc.sync.dma_start(bob4_sb[:], bob4[:, :])

            def proj_units(nt):
                # 6 small units (q, k, 4x v) for token block nt
                def qk_unit(w_sb, b_sb, dst, tag):
                    def u():
                        ps = ps_o.tile([128, QW], F32, tag=tag, name="ps_qk")
                        for c in range(4):
                            nc.tensor.matmul(
                                ps[:],
                                w_sb[:, 128 * c:128 * (c + 1)],
                                xt[c][:, QW * nt:QW * (nt + 1)],
                                start=(c == 0), stop=(c == 3),
                            )
                        nc.vector.tensor_scalar_add(
                            dst[:, QW * nt:QW * (nt + 1)], ps[:], b_sb[:])
                    return u

                def v_unit(tb):
                    def u():
                        ps = ps_o.tile([128, QW], F32, tag=f"po{tb % 2}",
                                       name="ps_v")
                        for c in range(4):
                            nc.tensor.matmul(
                                ps[:, 0:130],
                                xt[c][:, 128 * tb:128 * (tb + 1)],
                                wv_sb[:, 130 * c:130 * (c + 1)],
                                start=(c == 0), stop=(c == 3),
                            )
                        nc.vector.tensor_add(
                            vaug[:, 130 * tb:130 * (tb + 1)], ps[:, 0:130],
                            bvb_sb[:])
                    return u

                return ([qk_unit(wq_sb, bq_sb, qT, "po0"),
                         qk_unit(wk_sb, bk_sb, kT, "po1")]
                        + [v_unit(tb) for tb in range(4 * nt, 4 * (nt + 1))])

            def gather_unit(tg):
                # rs -> sbuf, cast bf16 -> f32, write the output rows
                def u():
                    g = stpool.tile([128, D], BF16, tag="g", name="g")
                    nc.sync.dma_start(g[:], rss[tg][:, :])
                    gf = stpool.tile([128, D], F32, tag="gf", name="gf")
                    nc.vector.tensor_copy(gf[:], g[:])
                    nc.sync.dma_start(out_ext[128 * tg:128 * (tg + 1), :], gf[:])
                return u

            def att_jloop(t, units):
                nj = 4 * t + 4          # causal: k blocks 0 .. 4t+3
                o0 = ps_o.tile([65, QW], F32, tag="o0", name="o0")
                o1 = ps_o.tile([65, QW], F32, tag="o1", name="o1")
                n_units = len(units)
                emitted = 0
                for j in range(nj):
                    # causal: q columns < q0 are fully masked for this k block
                    q0 = max(0, 128 * (j - 4 * t))
                    s = ps_s.tile([128, 2 * QW], F32, tag="s", name="s")
                    for h in (0, 1):
                        nc.tensor.matmul(
                            s[:, QW * h + q0:QW * (h + 1)],
                            kT[64 * h:64 * (h + 1), 128 * j:128 * (j + 1)],
                            qT[64 * h:64 * (h + 1), QW * t + q0:QW * (t + 1)],
                            start=True, stop=True,
                        )
                    p = ppool.tile([128, 2 * QW], BF16, tag="p", name="p")
                    if q0 == 0:
                        nc.scalar.activation(p[:], s[:], AF.Exp, bias=0.0, scale=0.125)
                    else:
                        sv = s[:].rearrange("k (h q) -> k h q", h=2)[:, :, q0:QW]
                        pv = p[:].rearrange("k (h q) -> k h q", h=2)[:, :, q0:QW]
                        nc.scalar.activation(pv, sv, AF.Exp, bias=0.0, scale=0.125)
                    while emitted < n_units and emitted <= j * n_units // nj:
                        units[emitted]()
                        emitted += 1
                    if j >= 4 * t:  # diagonal 128-col boundary: 0/1 mask (r=0 tile)
                        for h in (0, 1):
                            nc.vector.tensor_mul(
                                p[:, QW * h + q0:QW * h + q0 + 128],
                                p[:, QW * h + q0:QW * h + q0 + 128],
                                mask_sb[:, 0:128],
                            )
                    for h, oo in ((0, o0), (1, o1)):
                        nc.tensor.matmul(
                            oo[:, q0:QW],
                            vaug[:, 130 * j + 65 * h:130 * j + 65 * (h + 1)],
                            p[:, QW * h + q0:QW * (h + 1)],
                            start=(j == 0), stop=(j == nj - 1),
                        )
                for u in units[emitted:]:
                    u()
                return o0, o1

            def epilogue(t, o0, o1):
                # softmax denominators (row 64 of o0/o1), normalize, O-proj.
                # free the o accumulators fast: copy O^T (rows 0..63) to SBUF
                # and the l rows (row 64) to an SBUF staging row.
                oc0 = spool.tile([HD, QW], BF16, tag="oc0", name="oc0")
                oc1 = spool.tile([HD, QW], BF16, tag="oc1", name="oc1")
                nc.vector.tensor_copy(oc0[:], o0[0:64, :])
                nc.vector.tensor_copy(oc1[:], o1[0:64, :])
                lrow = spool.tile([128, 2 * QW], F32, tag="lrow", name="lrow")
                nc.vector.tensor_copy(lrow[64:65, 0:QW], o0[64:65, :])
                nc.vector.tensor_copy(lrow[64:65, QW:2 * QW], o1[64:65, :])

                # l -> natural per-token-partition layout [128, tb] via DMA,
                # then reciprocal (base-0 only: reciprocal_approx_fast quirk)
                l_nat = spool.tile([128, 8], F32, tag="l_nat", name="l_nat")
                nc.sync.dma_start(ldram[t][:].rearrange("(a q) -> a q", a=1),
                                  lrow[64:65, :])
                for h in range(2):
                    nc.sync.dma_start(
                        l_nat[:, 4 * h:4 * (h + 1)],
                        ldram[t][QW * h:QW * (h + 1)].rearrange(
                            "(tb p) -> p tb", p=128),
                    )
                linv_nat = spool.tile([128, 8], F32, tag="linv_nat", name="linv")
                nc.vector.reciprocal_approx_fast(linv_nat[:], l_nat[:])

                for tb in range(4):
                    po0 = ps_o.tile([128, QW], F32, tag="po0", name="po0")
                    po1 = ps_o.tile([128, QW], F32, tag="po1", name="po1")
                    nc.tensor.matmul(po0[:], oc0[:, 128 * tb:128 * (tb + 1)],
                                     wo0_sb[:], start=True, stop=True)
                    nc.tensor.matmul(po1[:], oc1[:, 128 * tb:128 * (tb + 1)],
                                     wo1_sb[:], start=True, stop=True)
                    tmp = stpool.tile([128, QW], F32, tag="tmp", name="tmp")
                    nc.vector.scalar_tensor_tensor(
                        tmp[:], po0[:], linv_nat[:, tb:tb + 1], bob4_sb[:],
                        mybir.AluOpType.mult, mybir.AluOpType.add)
                    st = stpool.tile([128, QW], BF16, tag="st", name="st")
                    nc.vector.scalar_tensor_tensor(
                        st[:], po1[:], linv_nat[:, 4 + tb:5 + tb], tmp[:],
                        mybir.AluOpType.mult, mybir.AluOpType.add)
                    nc.sync.dma_start(
                        parts[t][128 * tb:128 * (tb + 1), :], st[:]
                    )
                # chunked ReduceScatter: overlaps with later q tiles
                nc.gpsimd.collective_compute(
                    "ReduceScatter",
                    mybir.AluOpType.add,
                    replica_groups=[[0, 1, 2, 3], [4, 5, 6, 7]],
                    ins=[parts[t][:, :]],
                    outs=[rss[t][:, :]],
                )

            for u in proj_units(0):
                u()
            for t in range(NT):
                units = []
                if t >= 4:
                    units.append(gather_unit(t - 4))
                if t + 1 < NT:
                    units.extend(proj_units(t + 1))
                o0, o1 = att_jloop(t, units)
                epilogue(t, o0, o1)
            for tg in range(NT - 4, NT):
                gather_unit(tg)()

    nc.finalize()
    return nc


def _make_in_maps(x, Wqkv, bqkv, Wo, bo):
    # causal 0/1 multiplicative masks for the 4 diagonal sub-block offsets:
    # keep (p, o) where o >= 128*r + p  (k = 128*(4t+r)+p, q = 512*t+o)
    p_idx = np.arange(128)[:, None]
    o_idx = np.arange(QW)[None, :]
    maskc = np.concatenate(
        [(o_idx >= 128 * r + p_idx).astype(np.float32) for r in range(4)], axis=1
    ).astype(BF16_NP)

    in_maps = []
    for core in range(NCORES):
        b = core // 4
        g = core % 4
        rows = slice(128 * g, 128 * (g + 1))
        wq = Wqkv[0:D][rows]            # [128, 512]
        wk = Wqkv[D:2 * D][rows]
        wv = Wqkv[2 * D:3 * D][rows]
        wvT = np.zeros((D, 130), dtype=np.float32)
        wvT[:, 0:64] = wv[0:64].T
        wvT[:, 65:129] = wv[64:128].T
        bvb = np.zeros((128, 130), dtype=np.float32)
        bvb[:, 0:64] = bqkv[2 * D:3 * D][rows][0:64][None, :]
        bvb[:, 64] = 1.0
        bvb[:, 65:129] = bqkv[2 * D:3 * D][rows][64:128][None, :]
        bvb[:, 129] = 1.0
        in_maps.append({
            "xT": np.ascontiguousarray(x[b].T).astype(BF16_NP),
            "wqT": np.ascontiguousarray(wq.T).astype(BF16_NP),
            "wkT": np.ascontiguousarray(wk.T).astype(BF16_NP),
            "wvT": wvT.astype(BF16_NP),
            "bq": np.ascontiguousarray(bqkv[0:D][rows][:, None]).astype(np.float32),
            "bk": np.ascontiguousarray(bqkv[D:2 * D][rows][:, None]).astype(np.float32),
            "bvb": bvb,
            "wo0": np.ascontiguousarray(Wo[:, 128 * g:128 * g + 64].T).astype(BF16_NP),
            "wo1": np.ascontiguousarray(Wo[:, 128 * g + 64:128 * (g + 1)].T).astype(BF16_NP),
            "bob4": np.tile((bo / 4.0).astype(np.float32)[None, :], (128, 1)),
            "maskc": maskc,
        })
    return in_maps


def run(x, Wqkv, bqkv, Wo, bo, trace=False):
    if "nc" not in _CACHE:
        _CACHE["nc"] = _build_nc()
    nc = _CACHE["nc"]
    in_maps = _make_in_maps(x, Wqkv, bqkv, Wo, bo)
    res = run_bass_kernel_spmd(nc, in_maps, core_ids=list(range(NCORES)), trace=trace)
    out = np.empty((B, S, D), dtype=np.float32)
    for core in range(NCORES):
        b = core // 4
        r = core % 4
        o = res.results[core]["out"]
        # chunked ReduceScatter: rank r holds rows 512t+128r..+128 per q tile t
        for t in range(NT):
            out[b, QW * t + 128 * r:QW * t + 128 * (r + 1), :] = o[128 * t:128 * (t + 1)]
    return out, res


def kernel(x, Wqkv, bqkv, Wo, bo):
    out, _ = run(np.asarray(x, dtype=np.float32), np.asarray(Wqkv, dtype=np.float32),
                 np.asarray(bqkv, dtype=np.float32), np.asarray(Wo, dtype=np.float32),
                 np.asarray(bo, dtype=np.float32))
    return out


# revision 26
# speedup vs baseline: 1.1148x; 1.0586x over previous
"""Distributed causal-attention block (dense_transformer) on 8 TRN2 NeuronCores.

Sharding: data-parallel over batch (b=2) x tensor-parallel over head pairs
(8 heads -> 4 groups of 2). Core i handles batch i//4, heads (2*(i%4), 2*(i%4)+1).
Per-core: QKV projection for its 2 heads (transposed layouts so attention is
transpose-free), block-causal flash-style attention (S^T = K @ Q^T formulation,
softmax denominator via an augmented ones-column in V), partial output
projection, then ReduceScatter(add) over each 4-core batch group.

Software pipelining: projections for q tile t+1 and the output gather for
tile t-4 are emitted as small work units inside tile t's k-block loop,
between the s matmul and PV — the tensor engine fills that slot while the
scalar engine runs exp, so neither engine starves at tile boundaries.

B, S, D, H = 2, 4096, 512, 8 (hd=64). Hardcoded per problem spec.
"""

import numpy as np
import ml_dtypes

import concourse.bacc as bacc
import concourse.mybir as mybir
from concourse import tile
from concourse.bass_utils import run_bass_kernel_spmd

B, S, D = 2, 4096, 512
H = 8
HD = D // H          # 64
NCORES = 8
R = 128              # qkv rows per core (2 heads x 64)
S4 = S // 4          # reduce-scatter shard rows
NT = 8               # q tiles of 512
QW = 512             # q tile width

BF16 = mybir.dt.bfloat16
F32 = mybir.dt.float32
AF = mybir.ActivationFunctionType
BF16_NP = ml_dtypes.bfloat16

_CACHE = {}


def _build_nc():
    nc = bacc.Bacc(num_devices=NCORES)

    xT = nc.declare_dram_parameter("xT", [D, S], BF16, isOutput=False)
    wqT = nc.declare_dram_parameter("wqT", [D, R], BF16, isOutput=False)
    wkT = nc.declare_dram_parameter("wkT", [D, R], BF16, isOutput=False)
    wvT = nc.declare_dram_parameter("wvT", [D, 130], BF16, isOutput=False)
    bq = nc.declare_dram_parameter("bq", [R, 1], F32, isOutput=False)
    bk = nc.declare_dram_parameter("bk", [R, 1], F32, isOutput=False)
    bvb = nc.declare_dram_parameter("bvb", [128, 130], F32, isOutput=False)
    wo0 = nc.declare_dram_parameter("wo0", [HD, D], BF16, isOutput=False)
    wo1 = nc.declare_dram_parameter("wo1", [HD, D], BF16, isOutput=False)
    bob4 = nc.declare_dram_parameter("bob4", [128, D], F32, isOutput=False)
    maskc = nc.declare_dram_parameter("maskc", [128, 4 * QW], BF16, isOutput=False)
    out_ext = nc.declare_dram_parameter("out", [S4, D], F32, isOutput=True)

    parts = [nc.dram_tensor(f"part{t}", [QW, D], BF16) for t in range(NT)]
    ldram = [nc.dram_tensor(f"ldram{t}", [2 * QW], F32) for t in range(NT)]
    rss = [nc.dram_tensor(f"rs{t}", [QW // 4, D], BF16) for t in range(NT)]

    with tile.TileContext(nc) as tc:
        with (
            tc.tile_pool(name="const", bufs=1) as cpool,
            tc.tile_pool(name="xres", bufs=1) as xpool,
            tc.tile_pool(name="pt", bufs=24) as ppool,
            tc.tile_pool(name="small", bufs=3) as spool,
            tc.tile_pool(name="stage", bufs=4) as stpool,
            tc.tile_pool(name="ps_s", bufs=2, space="PSUM") as ps_s,
            tc.tile_pool(name="ps_o", bufs=1, space="PSUM") as ps_o,
        ):
            # ---------- loads in first-use order: x first halves, QKV
            # weights, x second halves, then the epilogue constants
            xt = [xpool.tile([128, S], BF16, tag=f"xt{c}", name=f"xt{c}")
                  for c in range(4)]
            qT = xpool.tile([128, S], BF16, tag="qT")
            kT = xpool.tile([128, S], BF16, tag="kT")
            vaug = xpool.tile([128, 32 * 130], BF16, tag="vaug")

            def load_x(c, half):
                nc.sync.dma_start(
                    xt[c][:, (S // 2) * half:(S // 2) * (half + 1)],
                    xT[128 * c:128 * (c + 1),
                       (S // 2) * half:(S // 2) * (half + 1)],
                )

            for c in range(4):
                load_x(c, 0)
            wq_sb = cpool.tile([128, D], BF16)
            nc.sync.dma_start(wq_sb[:].rearrange("p (c m) -> p c m", c=4),
                              wqT[:, :].rearrange("(c p) m -> p c m", p=128))
            wk_sb = cpool.tile([128, D], BF16)
            nc.sync.dma_start(wk_sb[:].rearrange("p (c m) -> p c m", c=4),
                              wkT[:, :].rearrange("(c p) m -> p c m", p=128))
            wv_sb = cpool.tile([128, 4 * 130], BF16)
            nc.sync.dma_start(wv_sb[:].rearrange("p (c m) -> p c m", c=4),
                              wvT[:, :].rearrange("(c p) m -> p c m", p=128))
            bq_sb = cpool.tile([R, 1], F32)
            nc.sync.dma_start(bq_sb[:], bq[:, :])
            bk_sb = cpool.tile([R, 1], F32)
            nc.sync.dma_start(bk_sb[:], bk[:, :])
            bvb_sb = cpool.tile([128, 130], F32)
            nc.sync.dma_start(bvb_sb[:], bvb[:, :])
            mask_sb = cpool.tile([128, 4 * QW], BF16)
            nc.sync.dma_start(mask_sb[:], maskc[:, :])
            for c in range(4):
                load_x(c, 1)
            wo0_sb = cpool.tile([HD, D], BF16)
            nc.sync.dma_start(wo0_sb[:], wo0[:, :])
            wo1_sb = cpool.tile([HD, D], BF16)
            nc.sync.dma_start(wo1_sb[:], wo1[:, :])
            bob4_sb = cpool.tile([128, D], F32)
            nc.sync.dma_start(bob4_sb[:], bob4[:, :])

            def proj_units(nt):
                # 6 small units (q, k, 4x v) for token block nt
                def qk_unit(w_sb, b_sb, dst, tag):
                    def u():
                        ps = ps_o.tile([128, QW], F32, tag=tag, name="ps_qk")
                        for c in range(4):
                            nc.tensor.matmul(
                                ps[:],
                                w_sb[:, 128 * c:128 * (c + 1)],
                                xt[c][:, QW * nt:QW * (nt + 1)],
                                start=(c == 0), stop=(c == 3),
                            )
                        nc.vector.tensor_scalar_add(
                            dst[:, QW * nt:QW * (nt + 1)], ps[:], b_sb[:])
                    return u

                def v_unit(tb):
                    def u():
                        ps = ps_o.tile([128, QW], F32, tag=f"po{tb % 2}",
                                       name="ps_v")
                        for c in range(4):
                            nc.tensor.matmul(
                                ps[:, 0:130],
                                xt[c][:, 128 * tb:128 * (tb + 1)],
                                wv_sb[:, 130 * c:130 * (c + 1)],
                                start=(c == 0), stop=(c == 3),
                            )
                        nc.vector.tensor_add(
                            vaug[:, 130 * tb:130 * (tb + 1)], ps[:, 0:130],
                            bvb_sb[:])
                    return u

                return ([qk_unit(wq_sb, bq_sb, qT, "po0"),
                         qk_unit(wk_sb, bk_sb, kT, "po1")]
                        + [v_unit(tb) for tb in range(4 * nt, 4 * (nt + 1))])

            def gather_unit(tg):
                # rs -> sbuf, cast bf16 -> f32, write the output rows
                def u():
                    g = stpool.tile([128, D], BF16, tag="g", name="g")
                    nc.sync.dma_start(g[:], rss[tg][:, :])
                    gf = stpool.tile([128, D], F32, tag="gf", name="gf")
                    nc.vector.tensor_copy(gf[:], g[:])
                    nc.sync.dma_start(out_ext[128 * tg:128 * (tg + 1), :], gf[:])
                return u

            def att_jloop(t, units):
                nj = 4 * t + 4          # causal: k blocks 0 .. 4t+3
                o0 = ps_o.tile([65, QW], F32, tag="o0", name="o0")
                o1 = ps_o.tile([65, QW], F32, tag="o1", name="o1")
                n_units = len(units)
                emitted = 0
                for j in range(nj):
                    # causal: q columns < q0 are fully masked for this k block
                    q0 = max(0, 128 * (j - 4 * t))
                    s = ps_s.tile([128, 2 * QW], F32, tag="s", name="s")
                    for h in (0, 1):
                        nc.tensor.matmul(
                            s[:, QW * h + q0:QW * (h + 1)],
                            kT[64 * h:64 * (h + 1), 128 * j:128 * (j + 1)],
                            qT[64 * h:64 * (h + 1), QW * t + q0:QW * (t + 1)],
                            start=True, stop=True,
                        )
                    p = ppool.tile([128, 2 * QW], BF16, tag="p", name="p")
                    if q0 == 0:
                        nc.scalar.activation(p[:], s[:], AF.Exp, bias=0.0, scale=0.125)
                    else:
                        sv = s[:].rearrange("k (h q) -> k h q", h=2)[:, :, q0:QW]
                        pv = p[:].rearrange("k (h q) -> k h q", h=2)[:, :, q0:QW]
                        nc.scalar.activation(pv, sv, AF.Exp, bias=0.0, scale=0.125)
                    while emitted < n_units and emitted <= j * n_units // nj:
                        units[emitted]()
                        emitted += 1
                    if j >= 4 * t:  # diagonal 128-col boundary: 0/1 mask (r=0 tile)
                        for h in (0, 1):
                            nc.vector.tensor_mul(
                                p[:, QW * h + q0:QW * h + q0 + 128],
                                p[:, QW * h + q0:QW * h + q0 + 128],
                                mask_sb[:, 0:128],
                            )
                    for h, oo in ((0, o0), (1, o1)):
                        nc.tensor.matmul(
                            oo[:, q0:QW],
                            vaug[:, 130 * j + 65 * h:130 * j + 65 * (h + 1)],
                            p[:, QW * h + q0:QW * (h + 1)],
                            start=(j == 0), stop=(j == nj - 1),
                        )
                for u in units[emitted:]:
                    u()
                return o0, o1

            def epilogue_a(t, o0, o1):
                # Free the o accumulators fast: copy O^T (rows 0..63) and the
                # l rows (row 64) to SBUF, launch the l transpose DMA
                # round-trip. No waits on the DVE queue here.
                oc0 = spool.tile([HD, QW], BF16, tag="oc0", name="oc0")
                oc1 = spool.tile([HD, QW], BF16, tag="oc1", name="oc1")
                nc.vector.tensor_copy(oc0[:], o0[0:64, :])
                nc.vector.tensor_copy(oc1[:], o1[0:64, :])
                lrow = spool.tile([128, 2 * QW], F32, tag="lrow", name="lrow")
                nc.vector.tensor_copy(lrow[64:65, 0:QW], o0[64:65, :])
                nc.vector.tensor_copy(lrow[64:65, QW:2 * QW], o1[64:65, :])
                # l -> natural per-token-partition layout [128, tb] via DMA
                l_nat = spool.tile([128, 8], F32, tag="l_nat", name="l_nat")
                nc.sync.dma_start(ldram[t][:].rearrange("(a q) -> a q", a=1),
                                  lrow[64:65, :])
                for h in range(2):
                    nc.sync.dma_start(
                        l_nat[:, 4 * h:4 * (h + 1)],
                        ldram[t][QW * h:QW * (h + 1)].rearrange(
                            "(tb p) -> p tb", p=128),
                    )
                return oc0, oc1, l_nat

            def epilogue_b_units(t, oc0, oc1, l_nat):
                # Deferred into the NEXT tile's j-loop slack slots so the
                # reciprocal's wait on the l round-trip DMA never blocks the
                # DVE queue at a tile boundary.
                state = {}

                def tb_unit(tb):
                    def u():
                        if tb == 0:
                            linv = spool.tile([128, 8], F32, tag="linv_nat",
                                              name="linv")
                            # base-0 only: reciprocal_approx_fast quirk
                            nc.vector.reciprocal_approx_fast(linv[:], l_nat[:])
                            state["linv"] = linv
                        linv = state["linv"]
                        po0 = ps_o.tile([128, QW], F32, tag="po0", name="po0")
                        po1 = ps_o.tile([128, QW], F32, tag="po1", name="po1")
                        nc.tensor.matmul(po0[:], oc0[:, 128 * tb:128 * (tb + 1)],
                                         wo0_sb[:], start=True, stop=True)
                        nc.tensor.matmul(po1[:], oc1[:, 128 * tb:128 * (tb + 1)],
                                         wo1_sb[:], start=True, stop=True)
                        tmp = stpool.tile([128, QW], F32, tag="tmp", name="tmp")
                        nc.vector.scalar_tensor_tensor(
                            tmp[:], po0[:], linv[:, tb:tb + 1], bob4_sb[:],
                            mybir.AluOpType.mult, mybir.AluOpType.add)
                        st = stpool.tile([128, QW], BF16, tag="st", name="st")
                        nc.vector.scalar_tensor_tensor(
                            st[:], po1[:], linv[:, 4 + tb:5 + tb], tmp[:],
                            mybir.AluOpType.mult, mybir.AluOpType.add)
                        nc.sync.dma_start(
                            parts[t][128 * tb:128 * (tb + 1), :], st[:]
                        )
                        if tb == 3:
                            # chunked ReduceScatter: overlaps with later tiles
                            nc.gpsimd.collective_compute(
                                "ReduceScatter",
                                mybir.AluOpType.add,
                                replica_groups=[[0, 1, 2, 3], [4, 5, 6, 7]],
                                ins=[parts[t][:, :]],
                                outs=[rss[t][:, :]],
                            )
                    return u

                return [tb_unit(tb) for tb in range(4)]

            for u in proj_units(0):
                u()
            pend = []
            for t in range(NT):
                units = list(pend)
                if t >= 4:
                    units.append(gather_unit(t - 4))
                if t + 1 < NT:
                    units.extend(proj_units(t + 1))
                o0, o1 = att_jloop(t, units)
                pend = epilogue_b_units(t, *epilogue_a(t, o0, o1))
            for u in pend:      # tile 7's O-proj + ReduceScatter
                u()
            for tg in range(NT - 4, NT):
                gather_unit(tg)()

    nc.finalize()
    return nc


def _make_in_maps(x, Wqkv, bqkv, Wo, bo):
    # causal 0/1 multiplicative masks for the 4 diagonal sub-block offsets:
    # keep (p, o) where o >= 128*r + p  (k = 128*(4t+r)+p, q = 512*t+o)
    p_idx = np.arange(128)[:, None]
    o_idx = np.arange(QW)[None, :]
    maskc = np.concatenate(
        [(o_idx >= 128 * r + p_idx).astype(np.float32) for r in range(4)], axis=1
    ).astype(BF16_NP)

    in_maps = []
    for core in range(NCORES):
        b = core // 4
        g = core % 4
        rows = slice(128 * g, 128 * (g + 1))
        wq = Wqkv[0:D][rows]            # [128, 512]
        wk = Wqkv[D:2 * D][rows]
        wv = Wqkv[2 * D:3 * D][rows]
        wvT = np.zeros((D, 130), dtype=np.float32)
        wvT[:, 0:64] = wv[0:64].T
        wvT[:, 65:129] = wv[64:128].T
        bvb = np.zeros((128, 130), dtype=np.float32)
        bvb[:, 0:64] = bqkv[2 * D:3 * D][rows][0:64][None, :]
        bvb[:, 64] = 1.0
        bvb[:, 65:129] = bqkv[2 * D:3 * D][rows][64:128][None, :]
        bvb[:, 129] = 1.0
        in_maps.append({
            "xT": np.ascontiguousarray(x[b].T).astype(BF16_NP),
            "wqT": np.ascontiguousarray(wq.T).astype(BF16_NP),
            "wkT": np.ascontiguousarray(wk.T).astype(BF16_NP),
            "wvT": wvT.astype(BF16_NP),
            "bq": np.ascontiguousarray(bqkv[0:D][rows][:, None]).astype(np.float32),
            "bk": np.ascontiguousarray(bqkv[D:2 * D][rows][:, None]).astype(np.float32),
            "bvb": bvb,
            "wo0": np.ascontiguousarray(Wo[:, 128 * g:128 * g + 64].T).astype(BF16_NP),
            "wo1": np.ascontiguousarray(Wo[:, 128 * g + 64:128 * (g + 1)].T).astype(BF16_NP),
            "bob4": np.tile((bo / 4.0).astype(np.float32)[None, :], (128, 1)),
            "maskc": maskc,
        })
    return in_maps


def run(x, Wqkv, bqkv, Wo, bo, trace=False):
    if "nc" not in _CACHE:
        _CACHE["nc"] = _build_nc()
    nc = _CACHE["nc"]
    in_maps = _make_in_maps(x, Wqkv, bqkv, Wo, bo)
    res = run_bass_kernel_spmd(nc, in_maps, core_ids=list(range(NCORES)), trace=trace)
    out = np.empty((B, S, D), dtype=np.float32)
    for core in range(NCORES):
        b = core // 4
        r = core % 4
        o = res.results[core]["out"]
        # chunked ReduceScatter: rank r holds rows 512t+128r..+128 per q tile t
        for t in range(NT):
            out[b, QW * t + 128 * r:QW * t + 128 * (r + 1), :] = o[128 * t:128 * (t + 1)]
    return out, res


def kernel(x, Wqkv, bqkv, Wo, bo):
    out, _ = run(np.asarray(x, dtype=np.float32), np.asarray(Wqkv, dtype=np.float32),
                 np.asarray(bqkv, dtype=np.float32), np.asarray(Wo, dtype=np.float32),
                 np.asarray(bo, dtype=np.float32))
    return out
